# revision 18
# baseline (speedup 1.0000x reference)
"""ADTNLinear Trainium2 kernel.

Computes out = bias + sum_l permute(x, perms[l]) @ blockdiag(W[l]) for
x [4,4096,4096] f32, W [3,64,64,64], bias [4096], perms [3,4096] int64.

Strategy: data-parallel over the 16384 tokens across 8 NeuronCores (no
collectives).  Host pre-transposes each 2048-token shard to channel-major
x^T and casts to bf16.  Permuted channel rows reach SBUF two ways: a
host-permuted head (sublayer 0 fully; sublayers 1/2 for the first 4
iterations) streams contiguously via HWDGE from t=0, while gpsimd
dma_gather (one 4 KiB descriptor per channel row, indices carry the
permutation) covers the rest once its Q7 library is loaded.  TensorE
accumulates the three sublayers into PSUM per 128-channel group pair,
VectorE evacuates with a fused bias add (casting to bf16), and HWDGE
writes the channel-major output back.  Host upcasts, re-transposes and
re-assembles the full f32 output.
"""

from contextlib import ExitStack

import ml_dtypes
import numpy as np

import concourse.bacc as bacc
import concourse.bass as bass
import concourse.mybir as mybir
from concourse.library_config import mlp

NCORES = 8
B, S, C = 4, 4096, 4096
TOK = B * S            # 16384 tokens total
TPC = TOK // NCORES    # 2048 tokens per core (= dma_gather elem_size)
NPAIR = 32             # pairs of 64-channel groups (128 channels each)
PB = 4                 # pairs per block (one gather covers PB*128 rows)
NB = NPAIR // PB       # 8 iterations
NHEAD = 4              # iterations with host-permuted l=1,2 tiles
L = 3                  # sublayers
HALF = 512             # matmul N (one PSUM bank of f32)
NH = TPC // HALF       # 4 half-tiles per pair
IDXCOL = C // 16       # 256 wrapped-index columns per l

BF16 = mybir.dt.bfloat16
F32 = mybir.dt.float32
I16 = mybir.dt.int16

_CACHED_NC = None


def build_nc():
    nc = bacc.Bacc("TRN2")

    xt = nc.declare_dram_parameter("xt", [C, TPC], BF16, isOutput=False)
    # host-permuted heads: xt0 row j = x^T[perm0[j]] (all of sublayer 0);
    # xh[l-1] row j = x^T[perm_l[j]] for j < NHEAD*PB*128 (sublayers 1, 2)
    xt0 = nc.declare_dram_parameter("xt0", [C, TPC], BF16, isOutput=False)
    xh = nc.declare_dram_parameter("xh", [2 * NHEAD * PB * 128, TPC], BF16, isOutput=False)
    wp = nc.declare_dram_parameter("wp", [128, L * NPAIR * 128], BF16, isOutput=False)
    idx = nc.declare_dram_parameter("idx", [128, L * IDXCOL], I16, isOutput=False)
    br = nc.declare_dram_parameter("biasr", [128, NPAIR], F32, isOutput=False)
    out = nc.declare_dram_parameter("out", [C, TPC], BF16, isOutput=True)

    with ExitStack() as ctx:
        ec = ctx.enter_context
        # x^T tiles: slots [a(2) for l=0 | 2 + 2*s + (l-1) for s(3), l in {1,2}]
        NSLOT = 2 + 3 * 2
        xg = ec(nc.sbuf_tensor("xg", [128, NSLOT * PB * TPC], BF16))
        # [buf(2), pair_slot(PB), TPC] bf16 output staging
        ost = ec(nc.sbuf_tensor("ost", [128, 2 * PB * TPC], BF16))
        wsb = ec(nc.sbuf_tensor("wsb", [128, L * NPAIR * 128], BF16))
        isb = ec(nc.sbuf_tensor("isb", [128, L * IDXCOL], I16))
        bsb = ec(nc.sbuf_tensor("bsb", [128, NPAIR], F32))
        # PSUM: pair-parity k gets banks [4k, 4k+4) (one per half-tile)
        psum = [ec(nc.psum_tensor(f"ps{j}", [128, HALF], F32)) for j in range(8)]

        ld_g = ec(nc.semaphore("ld_g"))
        wsem = ec(nc.semaphore("wsem"))
        bsem = ec(nc.semaphore("bsem"))
        # rotating DMA sems: DMA incs arrive as 16 interleaved +1s, so a
        # threshold is only sound when it covers ALL DMAs issued on that sem
        # SWDGE (dma_gather) sems must never be shared with HWDGE loads.
        g0sem = [ec(nc.semaphore(f"g0_{par}")) for par in range(2)]
        hsem = [
            [ec(nc.semaphore(f"h{l}_{par}")) for par in range(2)] for l in (1, 2)
        ]
        gsem = [
            [ec(nc.semaphore(f"g{l}_{s3}")) for s3 in range(3)] for l in (1, 2)
        ]
        odsem = [ec(nc.semaphore(f"od{par}")) for par in range(2)]
        mm_sem = ec(nc.semaphore("mm_sem"))
        ev_sem = ec(nc.semaphore("ev_sem"))

        block = ec(nc.Block())

        def xg_slot(i, l):
            return i % 2 if l == 0 else 2 + 2 * (i % 3) + (l - 1)

        def xg_dst(i, l):
            base = xg_slot(i, l) * PB * TPC
            return xg[:, base : base + PB * TPC].rearrange("p (s n) -> p s n", n=TPC)

        def g_count(i):
            # number of l>0 gathers on slot (i%3) with index in [NHEAD, i]
            return sum(1 for k in range(NHEAD, i + 1) if k % 3 == i % 3)

        @block.gpsimd
        def _(g):
            g.load_library(mlp)
            g.dma_start(out=isb[:], in_=idx[:]).then_inc(ld_g, 16)
            g.wait_ge(ld_g, 16)
            for i in range(NHEAD, NB):
                if i >= 3:
                    # WAR: slot (i%3) was last written for iteration i-3;
                    # wait for that iteration's matmuls
                    g.wait_ge(mm_sem, 4 * PB * (i - 2))
                for l in (1, 2):
                    col0 = l * IDXCOL + (PB * 128 // 16) * i
                    g.dma_gather(
                        xg_dst(i, l),
                        xt[:],
                        isb[:, col0 : col0 + PB * 128 // 16],
                        PB * 128,
                        PB * 128,
                        TPC,
                    ).then_inc(gsem[l - 1][i % 3], 16)

        @block.scalar
        def _(sc):
            x0v = xt0[:].rearrange("(pb s p) n -> pb p s n", p=128, s=PB)
            xhv = xh[:].rearrange("(l pb s p) n -> l pb p s n", p=128, s=PB, pb=NHEAD)
            for i in range(NB):
                if i >= 2:
                    sc.wait_ge(mm_sem, 4 * PB * (i - 1))
                sc.dma_start(out=xg_dst(i, 0), in_=x0v[i]).then_inc(
                    g0sem[i % 2], 16
                )
                if i < NHEAD:
                    for l in (1, 2):
                        sc.dma_start(out=xg_dst(i, l), in_=xhv[l - 1, i]).then_inc(
                            hsem[l - 1][i % 2], 16
                        )

        @block.tensor
        def _(te):
            te.wait_ge(wsem, 16)
            for i in range(NB):
                for p in range(PB):
                    q = PB * i + p          # global pair sequence index
                    for l in range(L):
                        if p == 0:
                            if l == 0:
                                te.wait_ge(g0sem[i % 2], 16 * (i // 2 + 1))
                            elif i < NHEAD:
                                te.wait_ge(hsem[l - 1][i % 2], 16 * (i // 2 + 1))
                            else:
                                te.wait_ge(gsem[l - 1][i % 3], 16 * g_count(i))
                        lhsT = wsb[
                            :, (l * NPAIR + q) * 128 : (l * NPAIR + q + 1) * 128
                        ]
                        for h in range(NH):
                            j = (p % 2) * NH + h
                            if l == 0 and q >= 2:
                                # WAR: pair q-2's evac of this bank done
                                te.wait_ge(ev_sem, NH * (q - 2) + h + 1)
                            rbase = (xg_slot(i, l) * PB + p) * TPC + h * HALF
                            mm = te.matmul(
                                psum[j][:, :],
                                lhsT,
                                xg[:, rbase : rbase + HALF],
                                start=(l == 0),
                                stop=(l == L - 1),
                            )
                            if l == L - 1:
                                mm.then_inc(mm_sem, 1)

        @block.vector
        def _(v):
            v.wait_ge(bsem, 16)
            for i in range(NB):
                buf = i % 2
                if i >= 2:
                    # WAR: iteration i-2's output DMAs (which read this buf) done
                    v.wait_ge(odsem[i % 2], 16 * PB * (i // 2))
                for p in range(PB):
                    q = PB * i + p
                    for h in range(NH):
                        j = (p % 2) * NH + h
                        v.wait_ge(mm_sem, NH * q + h + 1)
                        ob = (buf * PB + p) * TPC + h * HALF
                        v.tensor_scalar_add(
                            ost[:, ob : ob + HALF], psum[j][:, :], bsb[:, q : q + 1]
                        ).then_inc(ev_sem, 1)

        @block.sync
        def _(sy):
            sy.dma_start(out=wsb[:], in_=wp[:]).then_inc(wsem, 16)
            sy.dma_start(out=bsb[:], in_=br[:]).then_inc(bsem, 16)
            ov = out[:].rearrange("(pb s p) n -> pb s p n", p=128, s=PB)
            for i in range(NB):
                buf = i % 2
                for p in range(PB):
                    sy.wait_ge(ev_sem, NH * (PB * i + p) + NH)
                    ob = (buf * PB + p) * TPC
                    sy.dma_start(
                        out=ov[i, p], in_=ost[:, ob : ob + TPC]
                    ).then_inc(odsem[i % 2], 16)
            sy.wait_ge(odsem[0], 16 * PB * (NB // 2))
            sy.wait_ge(odsem[1], 16 * PB * (NB // 2))

    nc.compile()
    return nc


def _prep_shared(W, bias, perms):
    """Host-side weight/index/bias rearrangement (identical on all cores)."""
    bf16 = ml_dtypes.bfloat16
    W = np.asarray(W, dtype=np.float32)
    W2 = W.reshape(L, NPAIR, 2, 64, 64)
    wpad = np.zeros((L, NPAIR, 128, 128), np.float32)
    wpad[:, :, :64, :64] = W2[:, :, 0]
    wpad[:, :, 64:, 64:] = W2[:, :, 1]
    # [k, l, pair, m] -> [128, L*NPAIR*128]
    wp = np.ascontiguousarray(
        wpad.transpose(2, 0, 1, 3).reshape(128, L * NPAIR * 128)
    ).astype(bf16)

    perms = np.asarray(perms).astype(np.int64)
    idxbuf = np.zeros((128, L * IDXCOL), np.int16)
    for l in range(L):
        vals = perms[l].astype(np.int16)                 # row = channel
        w16 = vals.reshape(IDXCOL, 16).T                 # wrapped in 16 parts
        idxbuf[:, l * IDXCOL : (l + 1) * IDXCOL] = np.tile(w16, (8, 1))

    biasr = np.ascontiguousarray(
        np.asarray(bias, dtype=np.float32).reshape(NPAIR, 128).T
    )
    return wp, idxbuf, biasr


def make_in_maps(x, W, bias, perms):
    bf16 = ml_dtypes.bfloat16
    wp, idxbuf, biasr = _prep_shared(W, bias, perms)
    xt_all = np.asarray(x, dtype=np.float32).reshape(TOK, C)
    perms = np.asarray(perms).astype(np.int64)
    nhrows = NHEAD * PB * 128
    in_maps = []
    for s in range(NCORES):
        shard = xt_all[s * TPC : (s + 1) * TPC]              # [TPC, C]
        xts = np.ascontiguousarray(shard.T).astype(bf16)     # [C, TPC]
        xt0 = np.ascontiguousarray(xts[perms[0]])            # l=0 pre-permuted
        xhv = np.ascontiguousarray(
            np.concatenate([xts[perms[1][:nhrows]], xts[perms[2][:nhrows]]], 0)
        )
        in_maps.append(
            {"xt": xts, "xt0": xt0, "xh": xhv, "wp": wp, "idx": idxbuf,
             "biasr": biasr}
        )
    return in_maps


def assemble_out(per_core_outs):
    out = np.empty((TOK, C), np.float32)
    for s in range(NCORES):
        out[s * TPC : (s + 1) * TPC] = per_core_outs[s].astype(np.float32).T
    return out.reshape(B, S, C)


def kernel(x, W, bias, perms):
    global _CACHED_NC
    from concourse.bass_utils import run_bass_kernel_spmd

    if _CACHED_NC is None:
        _CACHED_NC = build_nc()
    nc = _CACHED_NC
    in_maps = make_in_maps(x, W, bias, perms)
    res = run_bass_kernel_spmd(nc, in_maps, core_ids=list(range(NCORES)))
    return assemble_out([res.results[s]["out"] for s in range(NCORES)])


# revision 19
# speedup vs baseline: 1.0123x; 1.0123x over previous
"""ADTNLinear Trainium2 kernel.

Computes out = bias + sum_l permute(x, perms[l]) @ blockdiag(W[l]) for
x [4,4096,4096] f32, W [3,64,64,64], bias [4096], perms [3,4096] int64.

Strategy: data-parallel over the 16384 tokens across 8 NeuronCores (no
collectives).  Host pre-transposes each 2048-token shard to channel-major
x^T and casts to bf16.  Permuted channel rows reach SBUF two ways: a
host-permuted head (sublayer 0 fully; sublayers 1/2 for the first 4
iterations) streams contiguously via HWDGE from t=0, while gpsimd
dma_gather (one 4 KiB descriptor per channel row, indices carry the
permutation) covers the rest once its Q7 library is loaded.  TensorE
accumulates the three sublayers into PSUM per 128-channel group pair,
VectorE evacuates with a fused bias add (casting to bf16), and HWDGE
writes the channel-major output back.  Host upcasts, re-transposes and
re-assembles the full f32 output.
"""

from contextlib import ExitStack

import ml_dtypes
import numpy as np

import concourse.bacc as bacc
import concourse.bass as bass
import concourse.mybir as mybir
from concourse.library_config import mlp

NCORES = 8
B, S, C = 4, 4096, 4096
TOK = B * S            # 16384 tokens total
TPC = TOK // NCORES    # 2048 tokens per core (= dma_gather elem_size)
NPAIR = 32             # pairs of 64-channel groups (128 channels each)
PB = 4                 # pairs per block (one gather covers PB*128 rows)
NB = NPAIR // PB       # 8 iterations
NHEAD = 5              # iterations with host-permuted l=1,2 tiles
L = 3                  # sublayers
HALF = 512             # matmul N (one PSUM bank of f32)
NH = TPC // HALF       # 4 half-tiles per pair
IDXCOL = C // 16       # 256 wrapped-index columns per l

BF16 = mybir.dt.bfloat16
F32 = mybir.dt.float32
I16 = mybir.dt.int16

_CACHED_NC = None


def build_nc():
    nc = bacc.Bacc("TRN2")

    xt = nc.declare_dram_parameter("xt", [C, TPC], BF16, isOutput=False)
    # host-permuted heads: xt0 row j = x^T[perm0[j]] (all of sublayer 0);
    # xh[l-1] row j = x^T[perm_l[j]] for j < NHEAD*PB*128 (sublayers 1, 2)
    xt0 = nc.declare_dram_parameter("xt0", [C, TPC], BF16, isOutput=False)
    xh = nc.declare_dram_parameter("xh", [2 * NHEAD * PB * 128, TPC], BF16, isOutput=False)
    wp = nc.declare_dram_parameter("wp", [128, L * NPAIR * 128], BF16, isOutput=False)
    idx = nc.declare_dram_parameter("idx", [128, L * IDXCOL], I16, isOutput=False)
    br = nc.declare_dram_parameter("biasr", [128, NPAIR], F32, isOutput=False)
    out = nc.declare_dram_parameter("out", [C, TPC], BF16, isOutput=True)

    with ExitStack() as ctx:
        ec = ctx.enter_context
        # x^T tiles: slots [a(2) for l=0 | 2 + 2*s + (l-1) for s(3), l in {1,2}]
        NSLOT = 2 + 3 * 2
        xg = ec(nc.sbuf_tensor("xg", [128, NSLOT * PB * TPC], BF16))
        # [buf(2), pair_slot(PB), TPC] bf16 output staging
        ost = ec(nc.sbuf_tensor("ost", [128, 2 * PB * TPC], BF16))
        wsb = ec(nc.sbuf_tensor("wsb", [128, L * NPAIR * 128], BF16))
        isb = ec(nc.sbuf_tensor("isb", [128, L * IDXCOL], I16))
        bsb = ec(nc.sbuf_tensor("bsb", [128, NPAIR], F32))
        # PSUM: pair-parity k gets banks [4k, 4k+4) (one per half-tile)
        psum = [ec(nc.psum_tensor(f"ps{j}", [128, HALF], F32)) for j in range(8)]

        ld_g = ec(nc.semaphore("ld_g"))
        wsem = ec(nc.semaphore("wsem"))
        bsem = ec(nc.semaphore("bsem"))
        # rotating DMA sems: DMA incs arrive as 16 interleaved +1s, so a
        # threshold is only sound when it covers ALL DMAs issued on that sem
        # SWDGE (dma_gather) sems must never be shared with HWDGE loads.
        g0sem = [ec(nc.semaphore(f"g0_{par}")) for par in range(2)]
        hsem = [
            [ec(nc.semaphore(f"h{l}_{par}")) for par in range(2)] for l in (1, 2)
        ]
        gsem = [
            [ec(nc.semaphore(f"g{l}_{s3}")) for s3 in range(3)] for l in (1, 2)
        ]
        odsem = [ec(nc.semaphore(f"od{par}")) for par in range(2)]
        mm_sem = ec(nc.semaphore("mm_sem"))
        ev_sem = ec(nc.semaphore("ev_sem"))

        block = ec(nc.Block())

        def xg_slot(i, l):
            return i % 2 if l == 0 else 2 + 2 * (i % 3) + (l - 1)

        def xg_dst(i, l):
            base = xg_slot(i, l) * PB * TPC
            return xg[:, base : base + PB * TPC].rearrange("p (s n) -> p s n", n=TPC)

        def g_count(i):
            # number of l>0 gathers on slot (i%3) with index in [NHEAD, i]
            return sum(1 for k in range(NHEAD, i + 1) if k % 3 == i % 3)

        @block.gpsimd
        def _(g):
            g.load_library(mlp)
            g.dma_start(out=isb[:], in_=idx[:]).then_inc(ld_g, 16)
            g.wait_ge(ld_g, 16)
            for i in range(NHEAD, NB):
                if i >= 3:
                    # WAR: slot (i%3) was last written for iteration i-3;
                    # wait for that iteration's matmuls
                    g.wait_ge(mm_sem, 4 * PB * (i - 2))
                for l in (1, 2):
                    col0 = l * IDXCOL + (PB * 128 // 16) * i
                    g.dma_gather(
                        xg_dst(i, l),
                        xt[:],
                        isb[:, col0 : col0 + PB * 128 // 16],
                        PB * 128,
                        PB * 128,
                        TPC,
                    ).then_inc(gsem[l - 1][i % 3], 16)

        @block.scalar
        def _(sc):
            x0v = xt0[:].rearrange("(pb s p) n -> pb p s n", p=128, s=PB)
            xhv = xh[:].rearrange("(l pb s p) n -> l pb p s n", p=128, s=PB, pb=NHEAD)
            for i in range(NB):
                if i >= 2:
                    sc.wait_ge(mm_sem, 4 * PB * (i - 1))
                sc.dma_start(out=xg_dst(i, 0), in_=x0v[i]).then_inc(
                    g0sem[i % 2], 16
                )
                if i < NHEAD:
                    for l in (1, 2):
                        sc.dma_start(out=xg_dst(i, l), in_=xhv[l - 1, i]).then_inc(
                            hsem[l - 1][i % 2], 16
                        )

        @block.tensor
        def _(te):
            te.wait_ge(wsem, 16)
            for i in range(NB):
                for p in range(PB):
                    q = PB * i + p          # global pair sequence index
                    for l in range(L):
                        if p == 0:
                            if l == 0:
                                te.wait_ge(g0sem[i % 2], 16 * (i // 2 + 1))
                            elif i < NHEAD:
                                te.wait_ge(hsem[l - 1][i % 2], 16 * (i // 2 + 1))
                            else:
                                te.wait_ge(gsem[l - 1][i % 3], 16 * g_count(i))
                        lhsT = wsb[
                            :, (l * NPAIR + q) * 128 : (l * NPAIR + q + 1) * 128
                        ]
                        for h in range(NH):
                            j = (p % 2) * NH + h
                            if l == 0 and q >= 2:
                                # WAR: pair q-2's evac of this bank done
                                te.wait_ge(ev_sem, NH * (q - 2) + h + 1)
                            rbase = (xg_slot(i, l) * PB + p) * TPC + h * HALF
                            mm = te.matmul(
                                psum[j][:, :],
                                lhsT,
                                xg[:, rbase : rbase + HALF],
                                start=(l == 0),
                                stop=(l == L - 1),
                            )
                            if l == L - 1:
                                mm.then_inc(mm_sem, 1)

        @block.vector
        def _(v):
            v.wait_ge(bsem, 16)
            for i in range(NB):
                buf = i % 2
                if i >= 2:
                    # WAR: iteration i-2's output DMAs (which read this buf) done
                    v.wait_ge(odsem[i % 2], 16 * PB * (i // 2))
                for p in range(PB):
                    q = PB * i + p
                    for h in range(NH):
                        j = (p % 2) * NH + h
                        v.wait_ge(mm_sem, NH * q + h + 1)
                        ob = (buf * PB + p) * TPC + h * HALF
                        v.tensor_scalar_add(
                            ost[:, ob : ob + HALF], psum[j][:, :], bsb[:, q : q + 1]
                        ).then_inc(ev_sem, 1)

        @block.sync
        def _(sy):
            sy.dma_start(out=wsb[:], in_=wp[:]).then_inc(wsem, 16)
            sy.dma_start(out=bsb[:], in_=br[:]).then_inc(bsem, 16)
            ov = out[:].rearrange("(pb s p) n -> pb s p n", p=128, s=PB)
            for i in range(NB):
                buf = i % 2
                for p in range(PB):
                    sy.wait_ge(ev_sem, NH * (PB * i + p) + NH)
                    ob = (buf * PB + p) * TPC
                    sy.dma_start(
                        out=ov[i, p], in_=ost[:, ob : ob + TPC]
                    ).then_inc(odsem[i % 2], 16)
            sy.wait_ge(odsem[0], 16 * PB * (NB // 2))
            sy.wait_ge(odsem[1], 16 * PB * (NB // 2))

    nc.compile()
    return nc


def _prep_shared(W, bias, perms):
    """Host-side weight/index/bias rearrangement (identical on all cores)."""
    bf16 = ml_dtypes.bfloat16
    W = np.asarray(W, dtype=np.float32)
    W2 = W.reshape(L, NPAIR, 2, 64, 64)
    wpad = np.zeros((L, NPAIR, 128, 128), np.float32)
    wpad[:, :, :64, :64] = W2[:, :, 0]
    wpad[:, :, 64:, 64:] = W2[:, :, 1]
    # [k, l, pair, m] -> [128, L*NPAIR*128]
    wp = np.ascontiguousarray(
        wpad.transpose(2, 0, 1, 3).reshape(128, L * NPAIR * 128)
    ).astype(bf16)

    perms = np.asarray(perms).astype(np.int64)
    idxbuf = np.zeros((128, L * IDXCOL), np.int16)
    for l in range(L):
        vals = perms[l].astype(np.int16)                 # row = channel
        w16 = vals.reshape(IDXCOL, 16).T                 # wrapped in 16 parts
        idxbuf[:, l * IDXCOL : (l + 1) * IDXCOL] = np.tile(w16, (8, 1))

    biasr = np.ascontiguousarray(
        np.asarray(bias, dtype=np.float32).reshape(NPAIR, 128).T
    )
    return wp, idxbuf, biasr


def make_in_maps(x, W, bias, perms):
    bf16 = ml_dtypes.bfloat16
    wp, idxbuf, biasr = _prep_shared(W, bias, perms)
    xt_all = np.asarray(x, dtype=np.float32).reshape(TOK, C)
    perms = np.asarray(perms).astype(np.int64)
    nhrows = NHEAD * PB * 128
    in_maps = []
    for s in range(NCORES):
        shard = xt_all[s * TPC : (s + 1) * TPC]              # [TPC, C]
        xts = np.ascontiguousarray(shard.T).astype(bf16)     # [C, TPC]
        xt0 = np.ascontiguousarray(xts[perms[0]])            # l=0 pre-permuted
        xhv = np.ascontiguousarray(
            np.concatenate([xts[perms[1][:nhrows]], xts[perms[2][:nhrows]]], 0)
        )
        in_maps.append(
            {"xt": xts, "xt0": xt0, "xh": xhv, "wp": wp, "idx": idxbuf,
             "biasr": biasr}
        )
    return in_maps


def assemble_out(per_core_outs):
    out = np.empty((TOK, C), np.float32)
    for s in range(NCORES):
        out[s * TPC : (s + 1) * TPC] = per_core_outs[s].astype(np.float32).T
    return out.reshape(B, S, C)


def kernel(x, W, bias, perms):
    global _CACHED_NC
    from concourse.bass_utils import run_bass_kernel_spmd

    if _CACHED_NC is None:
        _CACHED_NC = build_nc()
    nc = _CACHED_NC
    in_maps = make_in_maps(x, W, bias, perms)
    res = run_bass_kernel_spmd(nc, in_maps, core_ids=list(range(NCORES)))
    return assemble_out([res.results[s]["out"] for s in range(NCORES)])
